# revision 1
# baseline (speedup 1.0000x reference)
"""Trainium2 Bass kernel for the pre-norm transformer block (nn_Block_54511724920843).

Sharding: data-parallel over the batch axis b (8 groups of 1024 tokens, one per
NeuronCore). Weights replicated. BatchNorm statistics span all 8192 tokens, so
per-core partial (sum, sumsq) are combined with a tiny AllReduce (2 of them:
one for BN1 over x, one for BN2 over x + attn_out).

On-chip layout is feature-major ("transposed"): activations are [feature, token]
so projections take the natural weight matrices as lhsT and per-feature BN
scale/shift become per-partition scalars. Attention math per head:
  scoresT[k, q] = kT_h^T-slice matmuls (K=dh=64), exp fused with mask+1/8 scale
  on the Scalar engine, AV accumulated in PSUM with a ones-column appended to v
  so softmax denominators fall out of the same matmuls (M=65, row 64 = sums).
Matmuls run in float32r (full PE rate, ~4e-5 rel err); the MLP runs in bf16.
"""
import sys

sys.path.insert(0, "/opt/trn_rl_repo")

import numpy as np
import ml_dtypes

import concourse.bass as bass
import concourse.tile as tile
import concourse.mybir as mybir
from concourse import bacc
from concourse.bass_utils import run_bass_kernel_spmd

F32 = mybir.dt.float32
F32R = mybir.dt.float32r
BF16 = mybir.dt.bfloat16
AF = mybir.ActivationFunctionType
ALU = mybir.AluOpType

N_CORES = 8
B, GS, ED = 8, 1024, 512
H = 8
DH = ED // H            # 64
TOK = GS                # tokens per core
NT = ED // 128          # 4 feature tiles
NH = ED * 4 // 128      # 16 hidden tiles
NC_TOK = TOK // 128     # 8 token chunks
EPS = 1e-5
N_TOTAL = B * GS        # 8192


def _bn_partial(nc, pools, name, t, x_tile, local):
    """Per-tile local (sum, sumsq) into local[:, 2t:2t+2]."""
    small, statsp, dram = pools
    st = small.tile([128, 2, 6], F32, tag=f"st_{name}", name=f"st_{name}")
    nc.vector.bn_stats(out=st[:, 0, :], in_=x_tile[:, 0:512])
    nc.vector.bn_stats(out=st[:, 1, :], in_=x_tile[:, 512:1024])
    mv = small.tile([128, 2], F32, tag=f"mv_{name}", name=f"mv_{name}")
    nc.vector.bn_aggr(out=mv, in_=st)
    nc.vector.tensor_scalar(
        out=local[:, 2 * t : 2 * t + 1], in0=mv[:, 0:1],
        scalar1=float(TOK), scalar2=None, op0=ALU.mult)
    msq = small.tile([128, 1], F32, tag=f"msq_{name}", name=f"msq_{name}")
    nc.vector.tensor_mul(out=msq, in0=mv[:, 0:1], in1=mv[:, 0:1])
    vps = small.tile([128, 1], F32, tag=f"vps_{name}", name=f"vps_{name}")
    nc.vector.tensor_add(out=vps, in0=mv[:, 1:2], in1=msq)
    nc.vector.tensor_scalar(
        out=local[:, 2 * t + 1 : 2 * t + 2], in0=vps,
        scalar1=float(TOK), scalar2=None, op0=ALU.mult)


def _bn_device(nc, tc, pools, x_tiles, g_sb, be_sb, eps_sb, name,
               collectives=True, local=None):
    """Global BatchNorm scale/shift from per-core x_tiles (4x [128,1024] f32).

    Returns (s_all, t_all): [128, 4] f32, per-feature scale and shift such that
    bn(x) = x*s + t. Uses bn_stats locally + AllReduce of (sum, sumsq).
    If ``local`` is given, per-tile stats were already emitted via _bn_partial.
    """
    small, statsp, dram = pools
    if local is None:
        local = statsp.tile([128, 8], F32, tag=f"loc_{name}", name=f"loc_{name}")
        for t in range(NT):
            _bn_partial(nc, pools, name, t, x_tiles[t], local)

    bounce_in = dram.tile([128, 8], F32, tag=f"bi_{name}", name=f"bi_{name}")
    bounce_out = dram.tile([128, 8], F32, tag=f"bo_{name}", name=f"bo_{name}")
    nc.scalar.dma_start(out=bounce_in, in_=local)
    if collectives:
        nc.gpsimd.collective_compute(
            "AllReduce", ALU.add,
            replica_groups=[list(range(N_CORES))],
            ins=[bounce_in[:]], outs=[bounce_out[:]])
    else:
        nc.scalar.dma_start(out=bounce_out, in_=bounce_in[:])
    glob = statsp.tile([128, 8], F32, tag=f"gl_{name}", name=f"gl_{name}")
    nc.scalar.dma_start(out=glob, in_=bounce_out)

    s_all = statsp.tile([128, 4], F32, tag=f"s_{name}", name=f"s_{name}")
    t_all = statsp.tile([128, 4], F32, tag=f"t_{name}", name=f"t_{name}")
    inv_n = 1.0 / float(N_TOTAL)
    gv = glob.rearrange("p (t two) -> p two t", two=2)
    sums, sqs = gv[:, 0, :], gv[:, 1, :]
    m = small.tile([128, 4], F32, tag=f"m_{name}", name=f"m_{name}")
    nc.vector.tensor_scalar(out=m, in0=sums, scalar1=inv_n, scalar2=None,
                            op0=ALU.mult)
    msq = small.tile([128, 4], F32, tag=f"gmsq_{name}", name=f"gmsq_{name}")
    nc.vector.tensor_mul(out=msq, in0=m, in1=m)
    # var = sumsq/N - mean^2
    var = small.tile([128, 4], F32, tag=f"var_{name}", name=f"var_{name}")
    nc.vector.scalar_tensor_tensor(
        out=var, in0=sqs, scalar=inv_n, in1=msq, op0=ALU.mult, op1=ALU.subtract)
    # rstd = sqrt(1/(var + eps)); reciprocal on DVE (accurate), Sqrt table prewarmed
    vpe = small.tile([128, 4], F32, tag=f"vpe_{name}", name=f"vpe_{name}")
    nc.vector.tensor_scalar(out=vpe, in0=var, scalar1=EPS, scalar2=None,
                            op0=ALU.add)
    rec = small.tile([128, 4], F32, tag=f"rec_{name}", name=f"rec_{name}")
    nc.vector.reciprocal(out=rec, in_=vpe)
    rstd = small.tile([128, 4], F32, tag=f"rstd_{name}", name=f"rstd_{name}")
    nc.scalar.activation(out=rstd, in_=rec, func=AF.Sqrt)
    # s = g * rstd ; t = be - mean * s
    nc.vector.tensor_mul(out=s_all, in0=g_sb, in1=rstd)
    sn = small.tile([128, 4], F32, tag=f"sn_{name}", name=f"sn_{name}")
    nc.vector.tensor_mul(out=sn, in0=s_all, in1=m)
    nc.vector.tensor_sub(out=t_all, in0=be_sb, in1=sn)
    return s_all, t_all


def build(sim=False, collectives=True, n_devices=N_CORES, stop_after=None):
    nc = _build_graph(sim=sim, collectives=collectives, n_devices=n_devices,
                      stop_after=stop_after)
    nc.compile()
    return nc


def _build_graph(sim=False, collectives=True, n_devices=N_CORES, stop_after=None):
    from contextlib import ExitStack

    nc = bacc.Bacc("TRN2", target_bir_lowering=False, debug=False,
                   num_devices=n_devices)

    XT = nc.dram_tensor("xt", [NT, 128, TOK], F32, kind="ExternalInput")
    WQ = nc.dram_tensor("wq", [128, NT, 512], F32, kind="ExternalInput")
    WK = nc.dram_tensor("wk", [128, NT, 512], F32, kind="ExternalInput")
    WV = nc.dram_tensor("wv", [128, NT, 512], F32, kind="ExternalInput")
    WO = nc.dram_tensor("wo", [64, H, 512], F32, kind="ExternalInput")
    WM1 = nc.dram_tensor("wm1", [128, NT, 2048], BF16, kind="ExternalInput")
    WM2 = nc.dram_tensor("wm2", [128, NH, 512], BF16, kind="ExternalInput")
    BQ = nc.dram_tensor("bq", [128, 4], F32, kind="ExternalInput")
    BK = nc.dram_tensor("bk", [128, 4], F32, kind="ExternalInput")
    BV = nc.dram_tensor("bv", [128, 512], F32, kind="ExternalInput")
    BO = nc.dram_tensor("bo", [128, 4], F32, kind="ExternalInput")
    B1 = nc.dram_tensor("b1m", [128, 16], F32, kind="ExternalInput")
    B2 = nc.dram_tensor("b2m", [128, 4], F32, kind="ExternalInput")
    G1 = nc.dram_tensor("g1", [128, 4], F32, kind="ExternalInput")
    BE1 = nc.dram_tensor("be1", [128, 4], F32, kind="ExternalInput")
    G2 = nc.dram_tensor("g2", [128, 4], F32, kind="ExternalInput")
    BE2 = nc.dram_tensor("be2", [128, 4], F32, kind="ExternalInput")
    AM = nc.dram_tensor("am", [128, 8], F32, kind="ExternalInput")
    OUT = nc.dram_tensor("outt", [NT, 128, TOK], F32, kind="ExternalOutput")

    gelu_func = AF.Exp if sim else AF.Gelu

    with tile.TileContext(nc) as tc, ExitStack() as ctx:
        vec = ctx.enter_context(tc.tile_pool(name="vec", bufs=1))
        small = ctx.enter_context(tc.tile_pool(name="small", bufs=8))
        statsp = ctx.enter_context(tc.tile_pool(name="stats", bufs=1))
        dram = ctx.enter_context(tc.tile_pool(name="dram", bufs=1, space="DRAM"))
        mlpwp = ctx.enter_context(tc.tile_pool(name="mlpw", bufs=1))
        x2p = ctx.enter_context(tc.tile_pool(name="x2", bufs=1))

        def vload(name, dram_t, shape, dtype=F32):
            t = vec.tile(shape, dtype, tag=name, name=name)
            nc.sync.dma_start(out=t, in_=dram_t[:, :])
            return t

        x2_tiles = [x2p.tile([128, TOK], F32, tag=f"x2_{t}", name=f"x2_{t}") for t in range(NT)]

        def dump_out(tiles, cast=False):
            for t in range(NT):
                src_ap = tiles[t].bitcast(F32) if cast else tiles[t]
                nc.sync.dma_start(out=OUT[t, :, :], in_=src_ap)

        with ExitStack() as s1:
            xp = s1.enter_context(tc.tile_pool(name="xt", bufs=1))
            qkp = s1.enter_context(tc.tile_pool(name="qk", bufs=1))
            vap = s1.enter_context(tc.tile_pool(name="vaug", bufs=1))

            x_tiles = []
            for t in range(NT):
                xt = xp.tile([128, TOK], F32, tag=f"x_{t}", name=f"x_{t}")
                nc.sync.dma_start(out=xt, in_=XT[t, :, :])
                x_tiles.append(xt)

            q_tiles = [qkp.tile([128, TOK], F32R, tag=f"q_{t}", name=f"q_{t}") for t in range(NT)]
            k_tiles = [qkp.tile([128, TOK], F32R, tag=f"k_{t}", name=f"k_{t}") for t in range(NT)]
            v_aug = vap.tile([128, NC_TOK, H, DH + 1], F32R, tag="vaug", name="vaug")
            ones_sb = vec.tile([128, NC_TOK, H, 1], F32, tag="ones", name="ones")
            nc.vector.memset(ones_sb, 1.0)
            nc.vector.tensor_copy(out=v_aug[:, :, :, DH : DH + 1], in_=ones_sb)

            # ======== Phase 1: BN1 + QKV projections ========
            with ExitStack() as s2:
                wqp = s2.enter_context(tc.tile_pool(name="wqkv", bufs=1))
                h1p = s2.enter_context(tc.tile_pool(name="h1", bufs=1))
                pj = s2.enter_context(
                    tc.tile_pool(name="pj", bufs=3, space="PSUM"))

                g1_sb = vload("g1", G1, [128, 4])
                be1_sb = vload("be1", BE1, [128, 4])
                wq_sb = wqp.tile([128, NT, 512], F32R, tag="wq", name="wqs")
                nc.sync.dma_start(out=wq_sb, in_=WQ.bitcast(F32R)[:, :, :])
                wk_sb = wqp.tile([128, NT, 512], F32R, tag="wk", name="wks")
                nc.sync.dma_start(out=wk_sb, in_=WK.bitcast(F32R)[:, :, :])
                wv_sb = wqp.tile([128, NT, 512], F32R, tag="wv", name="wvs")
                nc.sync.dma_start(out=wv_sb, in_=WV.bitcast(F32R)[:, :, :])
                bq_sb = vload("bq", BQ, [128, 4])
                bk_sb = vload("bk", BK, [128, 4])
                bv_sb = vload("bv", BV, [128, 512])
                am_sb = vload("am", AM, [128, 8])
                bo_sb = vload("bo", BO, [128, 4])
                g2_sb = vload("g2", G2, [128, 4])
                be2_sb = vload("be2", BE2, [128, 4])
                b1_sb = vload("b1", B1, [128, 16])
                b2_sb = vload("b2", B2, [128, 4])
                eps_sb = vec.tile([128, 1], F32, tag="eps", name="eps")
                nc.vector.memset(eps_sb, EPS)

                s1v, t1v = _bn_device(nc, tc, (small, statsp, dram),
                                      x_tiles, g1_sb, be1_sb, eps_sb, "bn1",
                                      collectives=collectives)
                if stop_after == "bn1":
                    dump_out(x_tiles)
                    return nc

                warm = vec.tile([128, 1], F32, tag="warm", name="warm")
                nc.scalar.activation(out=warm, in_=eps_sb, func=AF.Exp)

                h1_tiles = []
                for t in range(NT):
                    h1 = h1p.tile([128, TOK], F32R, tag=f"h1_{t}", name=f"h1_{t}")
                    nc.vector.tensor_scalar(
                        out=h1, in0=x_tiles[t],
                        scalar1=s1v[:, t : t + 1], scalar2=t1v[:, t : t + 1],
                        op0=ALU.mult, op1=ALU.add)
                    h1_tiles.append(h1)

                # q/k: out tiles [128 feat, tok]; lhsT = W chunk, rhs = h1T
                for (w_sb, b_sb, dst) in ((wq_sb, bq_sb, q_tiles),
                                          (wk_sb, bk_sb, k_tiles)):
                    for o in range(NT):
                        for hf in range(2):
                            p = pj.tile([128, 512], F32, tag="pjq", name="pjq")
                            for k in range(NT):
                                nc.tensor.matmul(
                                    p,
                                    w_sb[:, k, o * 128 : (o + 1) * 128],
                                    h1_tiles[k][:, hf * 512 : (hf + 1) * 512],
                                    start=(k == 0), stop=(k == NT - 1))
                            nc.scalar.activation(
                                out=dst[o][:, hf * 512 : (hf + 1) * 512],
                                in_=p, func=AF.Identity,
                                bias=b_sb[:, o : o + 1], scale=1.0)

                # v natural: out [tok-chunk 128, 512 feat] -> v_aug slices
                for tt in range(NC_TOK):
                    p = pj.tile([128, 512], F32, tag="pjv", name="pjv")
                    for k in range(NT):
                        nc.tensor.matmul(
                            p,
                            h1_tiles[k][:, tt * 128 : (tt + 1) * 128],
                            wv_sb[:, k, :],
                            start=(k == 0), stop=(k == NT - 1))
                    nc.vector.tensor_add(
                        out=v_aug[:, tt, :, 0:DH],
                        in0=p.rearrange("p (h d) -> p h d", h=H),
                        in1=bv_sb.rearrange("p (h d) -> p h d", h=H))

            if stop_after == "qkv":
                dump_out(q_tiles, cast=True)
                return nc

            otp = s1.enter_context(tc.tile_pool(name="ot", bufs=1))
            oT = otp.tile([64, H, TOK], F32R, tag="ot", name="ots")
            wop = s1.enter_context(tc.tile_pool(name="wo", bufs=1))
            wo_sb = wop.tile([64, H, 512], F32R, tag="wo", name="wos")
            nc.sync.dma_start(out=wo_sb, in_=WO.bitcast(F32R)[:, :, :])
            wm1_sb = mlpwp.tile([128, NT, 2048], BF16, tag="wm1", name="wm1s")
            nc.sync.dma_start(out=wm1_sb, in_=WM1[:, :, :])
            wm2_sb = mlpwp.tile([128, NH, 512], BF16, tag="wm2", name="wm2s")
            nc.sync.dma_start(out=wm2_sb, in_=WM2[:, :, :])

            # ======== Phase 2: attention (one head at a time) ========
            with ExitStack() as s3:
                scp = s3.enter_context(
                    tc.tile_pool(name="sc", bufs=2, space="PSUM"))
                avp = s3.enter_context(
                    tc.tile_pool(name="av", bufs=2, space="PSUM"))
                ep = s3.enter_context(tc.tile_pool(name="E", bufs=3))
                rp = s3.enter_context(tc.tile_pool(name="rec", bufs=1))

                for h in range(H):
                    t = h // 2
                    r = (h % 2) * 64
                    av = avp.tile([DH + 1, TOK], F32, tag="av", name="avs")
                    prev_e = None
                    for c in range(NC_TOK):
                        S = scp.tile([128, TOK], F32, tag="S", name="Ss")
                        for hf in range(2):
                            nc.tensor.matmul(
                                S[:, hf * 512 : (hf + 1) * 512],
                                k_tiles[t][r : r + 64, c * 128 : (c + 1) * 128],
                                q_tiles[t][r : r + 64, hf * 512 : (hf + 1) * 512],
                                start=True, stop=True)
                        E = ep.tile([128, TOK], F32R, tag="E", name="Es")
                        nc.scalar.activation(
                            out=E, in_=S, func=AF.Exp,
                            bias=am_sb[:, c : c + 1], scale=0.125)
                        if prev_e is not None:
                            pc, pe = prev_e
                            for hf in range(2):
                                nc.tensor.matmul(
                                    av[:, hf * 512 : (hf + 1) * 512],
                                    v_aug[:, pc, h, :],
                                    pe[:, hf * 512 : (hf + 1) * 512],
                                    start=(pc == 0), stop=(pc == NC_TOK - 1))
                        prev_e = (c, E)
                    pc, pe = prev_e
                    for hf in range(2):
                        nc.tensor.matmul(
                            av[:, hf * 512 : (hf + 1) * 512],
                            v_aug[:, pc, h, :],
                            pe[:, hf * 512 : (hf + 1) * 512],
                            start=(pc == 0), stop=(pc == NC_TOK - 1))
                    # softmax denominators live in row 64 of av
                    scr = rp.tile([128, TOK], F32, tag="scr", name="scrs")
                    nc.vector.reciprocal(out=scr[64:65, :], in_=av[64:65, :])
                    rec0 = rp.tile([1, TOK], F32, tag="rec0", name="rec0s")
                    nc.gpsimd.dma_start(out=rec0, in_=scr[64:65, :])
                    recb = rp.tile([64, TOK], F32, tag="recb", name="recbs")
                    nc.gpsimd.partition_broadcast(recb, rec0[0:1, :])
                    nc.vector.tensor_mul(
                        out=oT[:, h, :],
                        in0=av[0:64, :], in1=recb)

            if stop_after == "attn":
                dump_out(x_tiles)
                return nc

            warm2 = vec.tile([128, 1], F32, tag="warm2", name="warm2")
            nc.scalar.activation(out=warm2, in_=eps_sb, func=AF.Sqrt)

            # ======== Phase 3: output projection + residual ========
            with ExitStack() as s4:
                pop = s4.enter_context(
                    tc.tile_pool(name="po", bufs=4, space="PSUM"))
                bn2_local = statsp.tile([128, 8], F32, tag="loc_bn2",
                                        name="loc_bn2")
                for o in range(NT):
                    for hf in range(2):
                        p = pop.tile([128, 512], F32, tag="po", name="pos")
                        for h in range(H):
                            nc.tensor.matmul(
                                p,
                                wo_sb[:, h, o * 128 : (o + 1) * 128],
                                oT[:, h, hf * 512 : (hf + 1) * 512],
                                start=(h == 0), stop=(h == H - 1))
                        # x2 = (proj + bo) + x
                        nc.vector.scalar_tensor_tensor(
                            out=x2_tiles[o][:, hf * 512 : (hf + 1) * 512],
                            in0=p, scalar=bo_sb[:, o : o + 1],
                            in1=x_tiles[o][:, hf * 512 : (hf + 1) * 512],
                            op0=ALU.add, op1=ALU.add)
                    _bn_partial(nc, (small, statsp, dram), "bn2", o,
                                x2_tiles[o], bn2_local)

        if stop_after == "oproj":
            dump_out(x2_tiles)
            return nc

        # ======== Phase 4: BN2 + MLP ========
        with ExitStack() as s5:
            h2p = s5.enter_context(tc.tile_pool(name="h2", bufs=1))
            htp = s5.enter_context(tc.tile_pool(name="ht", bufs=1))
            outp = s5.enter_context(tc.tile_pool(name="outsb", bufs=2))
            pm1 = s5.enter_context(
                tc.tile_pool(name="pm1", bufs=2, space="PSUM"))
            pm2 = s5.enter_context(
                tc.tile_pool(name="pm2", bufs=4, space="PSUM"))

            s2v, t2v = _bn_device(nc, tc, (small, statsp, dram),
                                  x2_tiles, g2_sb, be2_sb, eps_sb, "bn2",
                                  collectives=collectives, local=bn2_local)
            warm3 = vec.tile([128, 1], F32, tag="warm3", name="warm3")
            nc.scalar.activation(out=warm3, in_=eps_sb, func=gelu_func)

            h2_tiles = []
            for t in range(NT):
                h2 = h2p.tile([128, TOK], BF16, tag=f"h2_{t}", name=f"h2_{t}")
                nc.vector.tensor_scalar(
                    out=h2, in0=x2_tiles[t],
                    scalar1=s2v[:, t : t + 1], scalar2=t2v[:, t : t + 1],
                    op0=ALU.mult, op1=ALU.add)
                h2_tiles.append(h2)

            ht = htp.tile([128, NH, TOK], BF16, tag="ht", name="hts")
            for o in range(NH):
                p = pm1.tile([128, TOK], F32, tag="pm1", name="pm1s")
                for hf in range(2):
                    for k in range(NT):
                        nc.tensor.matmul(
                            p[:, hf * 512 : (hf + 1) * 512],
                            wm1_sb[:, k, o * 128 : (o + 1) * 128],
                            h2_tiles[k][:, hf * 512 : (hf + 1) * 512],
                            start=(k == 0), stop=(k == NT - 1))
                nc.scalar.activation(
                    out=ht[:, o, :], in_=p, func=gelu_func,
                    bias=b1_sb[:, o : o + 1], scale=1.0)

            for o in range(NT):
                ot = outp.tile([128, TOK], F32, tag="osb", name="osbs")
                for hf in range(2):
                    p = pm2.tile([128, 512], F32, tag="pm2", name="pm2s")
                    for k in range(NH):
                        nc.tensor.matmul(
                            p,
                            wm2_sb[:, k, o * 128 : (o + 1) * 128],
                            ht[:, k, hf * 512 : (hf + 1) * 512],
                            start=(k == 0), stop=(k == NH - 1))
                    nc.vector.scalar_tensor_tensor(
                        out=ot[:, hf * 512 : (hf + 1) * 512],
                        in0=p, scalar=b2_sb[:, o : o + 1],
                        in1=x2_tiles[o][:, hf * 512 : (hf + 1) * 512],
                        op0=ALU.add, op1=ALU.add)
                nc.sync.dma_start(out=OUT[o, :, :], in_=ot)

    return nc


_NC_CACHE = {}


def _get_nc(sim=False):
    if sim not in _NC_CACHE:
        _NC_CACHE[sim] = build(sim=sim)
    return _NC_CACHE[sim]


def make_in_maps(x, mask, Wq, bq, Wk, bk, Wv, bv, Wo, bo, g1, be1, g2, be2,
                 W1, b1m, W2, b2m):
    """Host-side sharding + layout prep. Returns list of per-core input dicts."""
    xT = np.ascontiguousarray(x.T.astype(np.float32))          # [512, 8192]
    wq = np.ascontiguousarray(
        np.asarray(Wq, np.float32).reshape(NT, 128, 512).transpose(1, 0, 2))
    wk = np.ascontiguousarray(
        np.asarray(Wk, np.float32).reshape(NT, 128, 512).transpose(1, 0, 2))
    wv = np.ascontiguousarray(
        np.asarray(Wv, np.float32).reshape(NT, 128, 512).transpose(1, 0, 2))
    wo = np.ascontiguousarray(
        np.asarray(Wo, np.float32).reshape(H, 64, 512).transpose(1, 0, 2))
    wm1 = np.ascontiguousarray(
        np.asarray(W1, np.float32).reshape(NT, 128, 2048).transpose(1, 0, 2)
    ).astype(ml_dtypes.bfloat16)
    wm2 = np.ascontiguousarray(
        np.asarray(W2, np.float32).reshape(NH, 128, 512).transpose(1, 0, 2)
    ).astype(ml_dtypes.bfloat16)

    def pp(v, c):
        return np.ascontiguousarray(np.asarray(v, np.float32).reshape(c, 128).T)

    shared = {
        "wq": wq, "wk": wk, "wv": wv, "wo": wo, "wm1": wm1, "wm2": wm2,
        "bq": pp(bq, 4), "bk": pp(bk, 4), "bo": pp(bo, 4),
        "bv": np.ascontiguousarray(
            np.broadcast_to(np.asarray(bv, np.float32), (128, 512))),
        "b1m": pp(b1m, 16), "b2m": pp(b2m, 4),
        "g1": pp(g1, 4), "be1": pp(be1, 4), "g2": pp(g2, 4), "be2": pp(be2, 4),
    }
    am_full = np.where(np.asarray(mask, bool), 0.0, -1e9).astype(np.float32)
    in_maps = []
    for core in range(N_CORES):
        sl = xT[:, core * TOK : (core + 1) * TOK]
        m = dict(shared)
        m["xt"] = np.ascontiguousarray(sl.reshape(NT, 128, TOK))
        m["am"] = np.ascontiguousarray(am_full[core].reshape(8, 128).T)
        in_maps.append(m)
    return in_maps


_EXEC_CACHE = {}


def _get_executor():
    """Cached PJRT executor for the compiled kernel (same path
    run_bass_kernel_spmd takes under axon, but jitted once and reused)."""
    if "fn" in _EXEC_CACHE:
        return _EXEC_CACHE["fn"]
    import jax
    from jax.sharding import Mesh, PartitionSpec
    from jax.experimental.shard_map import shard_map
    import concourse.bass2jax as b2j

    nc = _get_nc(sim=False)
    b2j.install_neuronx_cc_hook()
    partition_name = (nc.partition_id_tensor.name
                      if nc.partition_id_tensor else None)
    in_names, out_names, out_avals, zero_outs = [], [], [], []
    for alloc in nc.m.functions[0].allocations:
        if not isinstance(alloc, mybir.MemoryLocationSet):
            continue
        name = alloc.memorylocations[0].name
        if alloc.kind == "ExternalInput":
            if name != partition_name:
                in_names.append(name)
        elif alloc.kind == "ExternalOutput":
            out_names.append(name)
            shape = tuple(alloc.tensor_shape)
            dtype = mybir.dt.np(alloc.dtype)
            out_avals.append(jax.core.ShapedArray(shape, dtype))
            zero_outs.append(np.zeros(shape, dtype))
    n_params = len(in_names)
    all_names = in_names + out_names
    if partition_name is not None:
        all_names = all_names + [partition_name]

    def _body(*args):
        operands = list(args)
        if partition_name is not None:
            operands.append(b2j.partition_id_tensor())
        return tuple(b2j._bass_exec_p.bind(
            *operands,
            out_avals=tuple(out_avals),
            in_names=tuple(all_names),
            out_names=tuple(out_names),
            lowering_input_output_aliases=(),
            sim_require_finite=True,
            sim_require_nnan=True,
            nc=nc,
        ))

    devices = jax.devices()[:N_CORES]
    mesh = Mesh(np.asarray(devices), ("core",))
    n_out = len(out_names)
    sharded = jax.jit(
        shard_map(_body, mesh=mesh,
                  in_specs=(PartitionSpec("core"),) * (n_params + n_out),
                  out_specs=(PartitionSpec("core"),) * n_out,
                  check_rep=False),
        keep_unused=True)

    def run(in_maps):
        per_core = [[np.asarray(m[nm]) for nm in in_names] for m in in_maps]
        concat_in = [
            np.concatenate([per_core[c][i] for c in range(N_CORES)], axis=0)
            for i in range(n_params)]
        concat_zeros = [
            np.zeros((N_CORES * z.shape[0], *z.shape[1:]), z.dtype)
            for z in zero_outs]
        out_arrs = sharded(*concat_in, *concat_zeros)
        return [
            {name: np.asarray(out_arrs[i]).reshape(
                N_CORES, *out_avals[i].shape)[c]
             for i, name in enumerate(out_names)}
            for c in range(N_CORES)]

    _EXEC_CACHE["fn"] = run
    return run


def gather_out(results):
    """results: list of per-core dicts with 'outt' [4, 128, 1024] -> [8192, 512]."""
    outs = []
    for core in range(N_CORES):
        oT = results[core]["outt"].reshape(ED, TOK)   # [512, 1024]
        outs.append(oT.T)                             # [1024, 512]
    return np.concatenate(outs, axis=0).astype(np.float32)


def kernel(**inputs) -> np.ndarray:
    inputs = dict(inputs)
    inputs.pop("b", None)
    inputs.pop("gs", None)
    in_maps = make_in_maps(**inputs)
    run = _get_executor()
    return gather_out(run(in_maps))



# revision 19
# speedup vs baseline: 7.4143x; 7.4143x over previous
"""Trainium2 Bass kernel for the pre-norm transformer block (nn_Block_54511724920843).

Sharding: data-parallel over the batch axis b (8 groups of 1024 tokens, one per
NeuronCore). Weights replicated.

v2 design notes (vs the earlier baseline):
- The first collective of a NEFF execution costs ~65us (ncfw warmup) while
  later ones cost ~5us. A throwaway AllReduce is fired at t=0 to absorb the
  warmup concurrently with input DMA + BN1 stats.
- BN1 statistics are computed REDUNDANTLY on every core from a replicated
  bf16 copy of the full x (DVE bn_stats over [128, 8192] tiles, ~35us),
  removing the BN1 AllReduce from the critical path entirely. Only BN2
  (which depends on the attention output) uses a (now warm) AllReduce.
- All matmuls run in bf16 (same PE cycle rate as f32r at N>=256, half the
  SBUF, FWL weight loads, lower power -> less HAM throttling).
- Softmax: scoresT[k, q] per head with exp fused on ScalarE (mask bias +
  1/8 scale); denominators ride row 64 of the AV matmul via a ones-column
  appended to v. 1/denom = exp(-ln d) on ScalarE (stays in the
  natural_log_exp_and_others table set -> no table switches), broadcast to
  64 partitions with a tiny ones-matmul, applied with one DVE multiply per
  head. BN rstd likewise uses exp(-0.5*ln(var+eps)).
- V bias is folded into the output-projection bias host-side
  (bo2 = bo + bv @ Wo, exact because softmax rows sum to 1).
"""
import sys

sys.path.insert(0, "/opt/trn_rl_repo")

import numpy as np
import ml_dtypes

import concourse.bass as bass
import concourse.tile as tile
import concourse.mybir as mybir
from concourse import bacc
from concourse.bass_utils import run_bass_kernel_spmd

F32 = mybir.dt.float32
F32R = mybir.dt.float32r
BF16 = mybir.dt.bfloat16
AF = mybir.ActivationFunctionType
ALU = mybir.AluOpType

N_CORES = 8
B, GS, ED = 8, 1024, 512
H = 8
DH = ED // H            # 64
TOK = GS                # tokens per core
NT = ED // 128          # 4 feature tiles
NH = ED * 4 // 128      # 16 hidden tiles
NC_TOK = TOK // 128     # 8 token chunks
EPS = 1e-5
N_TOTAL = B * GS        # 8192


def _rstd_lnexp(nc, small, name, var_ap, n_col):
    """rstd = exp(-0.5 * ln(var + eps)) — stays in the ln/exp table set."""
    vpe = small.tile([128, n_col], F32, tag=f"vpe_{name}", name=f"vpe_{name}")
    nc.vector.tensor_scalar(out=vpe, in0=var_ap, scalar1=EPS, scalar2=None,
                            op0=ALU.add)
    lnv = small.tile([128, n_col], F32, tag=f"lnv_{name}", name=f"lnv_{name}")
    nc.scalar.activation(out=lnv, in_=vpe, func=AF.Ln)
    rstd = small.tile([128, n_col], F32, tag=f"rst_{name}", name=f"rst_{name}")
    nc.scalar.activation(out=rstd, in_=lnv, func=AF.Exp, scale=-0.5)
    return rstd


def _bn_partial(nc, small, name, t, x_tile, local):
    """Per-tile local (sum, sumsq)*TOK into local[:, 2t:2t+2] (for the AR)."""
    st = small.tile([128, 2, 6], F32, tag=f"st_{name}", name=f"st_{name}")
    nc.vector.bn_stats(out=st[:, 0, :], in_=x_tile[:, 0:512])
    nc.vector.bn_stats(out=st[:, 1, :], in_=x_tile[:, 512:1024])
    mv = small.tile([128, 2], F32, tag=f"mv_{name}", name=f"mv_{name}")
    nc.vector.bn_aggr(out=mv, in_=st)
    nc.vector.tensor_scalar(
        out=local[:, 2 * t : 2 * t + 1], in0=mv[:, 0:1],
        scalar1=float(TOK), scalar2=None, op0=ALU.mult)
    msq = small.tile([128, 1], F32, tag=f"msq_{name}", name=f"msq_{name}")
    nc.vector.tensor_mul(out=msq, in0=mv[:, 0:1], in1=mv[:, 0:1])
    vps = small.tile([128, 1], F32, tag=f"vps_{name}", name=f"vps_{name}")
    nc.vector.tensor_add(out=vps, in0=mv[:, 1:2], in1=msq)
    nc.vector.tensor_scalar(
        out=local[:, 2 * t + 1 : 2 * t + 2], in0=vps,
        scalar1=float(TOK), scalar2=None, op0=ALU.mult)


def _bn_device(nc, pools, g_sb, be_sb, name, collectives, local):
    """Global BN scale/shift from per-core partial stats via one AllReduce."""
    small, statsp, dram = pools
    bounce_in = dram.tile([128, 8], F32, tag=f"bi_{name}", name=f"bi_{name}")
    bounce_out = dram.tile([128, 8], F32, tag=f"bo_{name}", name=f"bo_{name}")
    nc.scalar.dma_start(out=bounce_in, in_=local)
    if collectives:
        nc.gpsimd.collective_compute(
            "AllReduce", ALU.add,
            replica_groups=[list(range(N_CORES))],
            ins=[bounce_in[:]], outs=[bounce_out[:]])
    else:
        nc.scalar.dma_start(out=bounce_out, in_=bounce_in[:])
    glob = statsp.tile([128, 8], F32, tag=f"gl_{name}", name=f"gl_{name}")
    nc.scalar.dma_start(out=glob, in_=bounce_out)

    s_all = statsp.tile([128, 4], F32, tag=f"s_{name}", name=f"s_{name}")
    t_all = statsp.tile([128, 4], F32, tag=f"t_{name}", name=f"t_{name}")
    inv_n = 1.0 / float(N_TOTAL)
    gv = glob.rearrange("p (t two) -> p two t", two=2)
    sums, sqs = gv[:, 0, :], gv[:, 1, :]
    m = small.tile([128, 4], F32, tag=f"m_{name}", name=f"m_{name}")
    nc.vector.tensor_scalar(out=m, in0=sums, scalar1=inv_n, scalar2=None,
                            op0=ALU.mult)
    msq = small.tile([128, 4], F32, tag=f"gmsq_{name}", name=f"gmsq_{name}")
    nc.vector.tensor_mul(out=msq, in0=m, in1=m)
    var = small.tile([128, 4], F32, tag=f"var_{name}", name=f"var_{name}")
    nc.vector.scalar_tensor_tensor(
        out=var, in0=sqs, scalar=inv_n, in1=msq, op0=ALU.mult, op1=ALU.subtract)
    rstd = _rstd_lnexp(nc, small, name, var, 4)
    nc.vector.tensor_mul(out=s_all, in0=g_sb, in1=rstd)
    sn = small.tile([128, 4], F32, tag=f"sn_{name}", name=f"sn_{name}")
    nc.vector.tensor_mul(out=sn, in0=s_all, in1=m)
    nc.vector.tensor_sub(out=t_all, in0=be_sb, in1=sn)
    return s_all, t_all


def build(sim=False, collectives=True, n_devices=N_CORES, stop_after=None):
    nc = _build_graph(sim=sim, collectives=collectives, n_devices=n_devices,
                      stop_after=stop_after)
    nc.compile()
    return nc


def _build_graph(sim=False, collectives=True, n_devices=N_CORES, stop_after=None):
    from contextlib import ExitStack

    nc = bacc.Bacc("TRN2", target_bir_lowering=False, debug=False,
                   num_devices=n_devices)

    XTB = nc.dram_tensor("xtb", [NT, 128, N_TOTAL], BF16, kind="ExternalInput")
    XT = nc.dram_tensor("xt", [NT, 128, TOK], BF16, kind="ExternalInput")
    WQ = nc.dram_tensor("wq", [128, NT, 512], BF16, kind="ExternalInput")
    WK = nc.dram_tensor("wk", [128, NT, 512], BF16, kind="ExternalInput")
    WV = nc.dram_tensor("wv", [128, NT, 512], BF16, kind="ExternalInput")
    WO = nc.dram_tensor("wo", [64, H, 512], BF16, kind="ExternalInput")
    WM1 = nc.dram_tensor("wm1", [128, NT, 2048], BF16, kind="ExternalInput")
    WM2 = nc.dram_tensor("wm2", [128, NH, 512], BF16, kind="ExternalInput")
    BQ = nc.dram_tensor("bq", [128, 4], F32, kind="ExternalInput")
    BK = nc.dram_tensor("bk", [128, 4], F32, kind="ExternalInput")
    BO2 = nc.dram_tensor("bo2", [128, 4], F32, kind="ExternalInput")
    B1 = nc.dram_tensor("b1m", [128, 16], F32, kind="ExternalInput")
    B2 = nc.dram_tensor("b2m", [128, 4], F32, kind="ExternalInput")
    G1 = nc.dram_tensor("g1", [128, 4], F32, kind="ExternalInput")
    BE1 = nc.dram_tensor("be1", [128, 4], F32, kind="ExternalInput")
    G2 = nc.dram_tensor("g2", [128, 4], F32, kind="ExternalInput")
    BE2 = nc.dram_tensor("be2", [128, 4], F32, kind="ExternalInput")
    AM = nc.dram_tensor("am", [128, 8], F32, kind="ExternalInput")
    OUT = nc.dram_tensor("outt", [NT, 128, TOK], F32, kind="ExternalOutput")

    gelu_func = AF.Exp if sim else AF.Gelu

    with tile.TileContext(nc) as tc, ExitStack() as ctx:
        vec = ctx.enter_context(tc.tile_pool(name="vec", bufs=1))
        small = ctx.enter_context(tc.tile_pool(name="small", bufs=8))
        statsp = ctx.enter_context(tc.tile_pool(name="stats", bufs=1))
        dram = ctx.enter_context(tc.tile_pool(name="dram", bufs=1, space="DRAM"))
        x2p = ctx.enter_context(tc.tile_pool(name="x2", bufs=1))

        # ---- throwaway AllReduce: absorbs the ~65us first-collective
        # warmup concurrently with input DMA + BN1 stats. Result unused.
        if collectives:
            zz = vec.tile([128, 8], F32, tag="zz", name="zz")
            nc.vector.memset(zz, 0.0)
            dum_in = dram.tile([128, 8], F32, tag="dum_i", name="dum_i")
            dum_out = dram.tile([128, 8], F32, tag="dum_o", name="dum_o")
            nc.scalar.dma_start(out=dum_in, in_=zz)
            nc.gpsimd.collective_compute(
                "AllReduce", ALU.add,
                replica_groups=[list(range(N_CORES))],
                ins=[dum_in[:]], outs=[dum_out[:]])
            dum_sb = vec.tile([128, 8], F32, tag="dum_s", name="dum_s")
            nc.scalar.dma_start(out=dum_sb, in_=dum_out)

        def vload(name, dram_t, shape, dtype=F32):
            t = vec.tile(shape, dtype, tag=name, name=name)
            nc.sync.dma_start(out=t, in_=dram_t[:, :])
            return t

        x2_tiles = [x2p.tile([128, TOK], F32, tag=f"x2_{t}", name=f"x2_{t}")
                    for t in range(NT)]
        # opened before s1 so it outlives s1 without breaking pool LIFO
        # order; tiles + DMAs are issued after the BN1 stats scope closes.
        mlpwp = ctx.enter_context(tc.tile_pool(name="mlpw", bufs=1))

        def dump_out(tiles, cast=False):
            for t in range(NT):
                src_ap = tiles[t].bitcast(F32) if cast else tiles[t]
                nc.sync.dma_start(out=OUT[t, :, :], in_=src_ap)

        with ExitStack() as s1:
            xp = s1.enter_context(tc.tile_pool(name="xt", bufs=1))

            # local shard (bf16) — residual + h1 source
            x_tiles = []
            for t in range(NT):
                xt = xp.tile([128, TOK], BF16, tag=f"x_{t}", name=f"x_{t}")
                nc.sync.dma_start(out=xt, in_=XT[t, :, :])
                x_tiles.append(xt)

            g1_sb = vload("g1", G1, [128, 4])
            be1_sb = vload("be1", BE1, [128, 4])
            am_sb = vload("am", AM, [128, 8])
            bq_sb = vload("bq", BQ, [128, 4])
            bk_sb = vload("bk", BK, [128, 4])
            bo2_sb = vload("bo2", BO2, [128, 4])
            g2_sb = vload("g2", G2, [128, 4])
            be2_sb = vload("be2", BE2, [128, 4])
            b1_sb = vload("b1", B1, [128, 16])
            b2_sb = vload("b2", B2, [128, 4])
            eps_sb = vec.tile([128, 1], F32, tag="eps", name="eps")
            nc.vector.memset(eps_sb, EPS)

            # ---- BN1: replicated global stats from the full bf16 x ----
            s1v = statsp.tile([128, 4], F32, tag="s1v", name="s1v")
            t1v = statsp.tile([128, 4], F32, tag="t1v", name="t1v")
            with ExitStack() as sb_scope:
                xbp = sb_scope.enter_context(tc.tile_pool(name="xtb", bufs=1))
                NG = N_TOTAL // 512   # bn_stats free dim is capped at 512
                HALF = N_TOTAL // 2
                for t in range(NT):
                    xb = xbp.tile([128, N_TOTAL], BF16, tag=f"xb_{t}",
                                  name=f"xb_{t}")
                    nc.sync.dma_start(out=xb[:, 0:HALF], in_=XTB[t, :, 0:HALF])
                    nc.sync.dma_start(out=xb[:, HALF:N_TOTAL],
                                      in_=XTB[t, :, HALF:N_TOTAL])
                    st = small.tile([128, NG, 6], F32, tag=f"s1s_{t}",
                                    name=f"s1s_{t}")
                    for g in range(NG):
                        nc.vector.bn_stats(out=st[:, g, :],
                                           in_=xb[:, g * 512 : (g + 1) * 512])
                    mv = small.tile([128, 2], F32, tag=f"s1m_{t}",
                                    name=f"s1m_{t}")
                    nc.vector.bn_aggr(out=mv, in_=st)
                    rstd = _rstd_lnexp(nc, small, f"bn1_{t}", mv[:, 1:2], 1)
                    nc.vector.tensor_mul(out=s1v[:, t : t + 1],
                                         in0=g1_sb[:, t : t + 1], in1=rstd)
                    sn = small.tile([128, 1], F32, tag=f"s1n_{t}",
                                    name=f"s1n_{t}")
                    nc.vector.tensor_mul(out=sn, in0=s1v[:, t : t + 1],
                                         in1=mv[:, 0:1])
                    nc.vector.tensor_sub(out=t1v[:, t : t + 1],
                                         in0=be1_sb[:, t : t + 1], in1=sn)
            if stop_after == "bn1":
                dump_out(x2_tiles)
                return nc

            # MLP weights (issued after the xtb pool closes; needed ~120us in)
            wm1_sb = mlpwp.tile([128, NT, 2048], BF16, tag="wm1", name="wm1s")
            nc.sync.dma_start(out=wm1_sb, in_=WM1[:, :, :])
            wm2_sb = mlpwp.tile([128, NH, 512], BF16, tag="wm2", name="wm2s")
            nc.sync.dma_start(out=wm2_sb, in_=WM2[:, :, :])
            wop = s1.enter_context(tc.tile_pool(name="wo", bufs=1))
            wo_sb = wop.tile([64, H, 512], BF16, tag="wo", name="wos")
            nc.sync.dma_start(out=wo_sb, in_=WO[:, :, :])

            qkp = s1.enter_context(tc.tile_pool(name="qk", bufs=1))
            vap = s1.enter_context(tc.tile_pool(name="vaug", bufs=1))
            q_tiles = [qkp.tile([128, TOK], BF16, tag=f"q_{t}", name=f"q_{t}")
                       for t in range(NT)]
            k_tiles = [qkp.tile([128, TOK], BF16, tag=f"k_{t}", name=f"k_{t}")
                       for t in range(NT)]
            v_aug = vap.tile([128, NC_TOK, H, DH + 1], BF16, tag="vaug",
                             name="vaug")
            ones_sb = vec.tile([128, NC_TOK, H, 1], F32, tag="ones",
                               name="ones")
            nc.vector.memset(ones_sb, 1.0)
            nc.vector.tensor_copy(out=v_aug[:, :, :, DH : DH + 1], in_=ones_sb)

            # ======== Phase 1: h1 + QKV projections (bf16) ========
            with ExitStack() as s2:
                wqp = s2.enter_context(tc.tile_pool(name="wqkv", bufs=1))
                h1p = s2.enter_context(tc.tile_pool(name="h1", bufs=1))
                pj = s2.enter_context(
                    tc.tile_pool(name="pj", bufs=3, space="PSUM"))

                wq_sb = wqp.tile([128, NT, 512], BF16, tag="wq", name="wqs")
                nc.sync.dma_start(out=wq_sb, in_=WQ[:, :, :])
                wk_sb = wqp.tile([128, NT, 512], BF16, tag="wk", name="wks")
                nc.sync.dma_start(out=wk_sb, in_=WK[:, :, :])
                wv_sb = wqp.tile([128, NT, 512], BF16, tag="wv", name="wvs")
                nc.sync.dma_start(out=wv_sb, in_=WV[:, :, :])

                h1_tiles = []
                for t in range(NT):
                    h1 = h1p.tile([128, TOK], BF16, tag=f"h1_{t}",
                                  name=f"h1_{t}")
                    nc.vector.tensor_scalar(
                        out=h1, in0=x_tiles[t],
                        scalar1=s1v[:, t : t + 1], scalar2=t1v[:, t : t + 1],
                        op0=ALU.mult, op1=ALU.add)
                    h1_tiles.append(h1)

                # q/k for tile 0 first so head 0 can start early, then v,
                # then the remaining q/k tiles.
                def qk_tile(w_sb, b_sb, dst, o):
                    for hf in range(2):
                        p = pj.tile([128, 512], F32, tag="pjq", name="pjq")
                        for k in range(NT):
                            nc.tensor.matmul(
                                p,
                                w_sb[:, k, o * 128 : (o + 1) * 128],
                                h1_tiles[k][:, hf * 512 : (hf + 1) * 512],
                                start=(k == 0), stop=(k == NT - 1))
                        nc.scalar.activation(
                            out=dst[o][:, hf * 512 : (hf + 1) * 512],
                            in_=p, func=AF.Identity,
                            bias=b_sb[:, o : o + 1], scale=1.0)

                qk_tile(wq_sb, bq_sb, q_tiles, 0)
                qk_tile(wk_sb, bk_sb, k_tiles, 0)

                # v natural: out [tok-chunk 128, 512 feat] -> v_aug (no bias;
                # bv is folded into bo2 host-side)
                for tt in range(NC_TOK):
                    p = pj.tile([128, 512], F32, tag="pjv", name="pjv")
                    for k in range(NT):
                        nc.tensor.matmul(
                            p,
                            h1_tiles[k][:, tt * 128 : (tt + 1) * 128],
                            wv_sb[:, k, :],
                            start=(k == 0), stop=(k == NT - 1))
                    nc.vector.tensor_copy(
                        out=v_aug[:, tt, :, 0:DH],
                        in_=p.rearrange("p (h d) -> p h d", h=H))

                for o in range(1, NT):
                    qk_tile(wq_sb, bq_sb, q_tiles, o)
                    qk_tile(wk_sb, bk_sb, k_tiles, o)

            if stop_after == "qkv":
                dump_out(q_tiles, cast=False)
                return nc

            # ======== Phase 2: attention ========
            otp = s1.enter_context(tc.tile_pool(name="ot", bufs=1))
            # row 64 carries the softmax denominators (one column block per
            # head); a single SBUF->SBUF DMA regathers them as [8, TOK].
            oT = otp.tile([DH + 1, H, TOK], F32, tag="ot", name="ots")
            oTn = otp.tile([64, H, TOK], BF16, tag="otn", name="otns")
            den_sb = otp.tile([8, TOK], F32, tag="den", name="dens")
            ones8f = vec.tile([1, 64], F32, tag="ones8f", name="ones8f")
            nc.vector.memset(ones8f, 1.0)
            ones8 = vec.tile([1, 64], BF16, tag="ones8", name="ones8")
            nc.vector.tensor_copy(out=ones8, in_=ones8f)

            with ExitStack() as s3:
                scp = s3.enter_context(
                    tc.tile_pool(name="sc", bufs=2, space="PSUM"))
                avp = s3.enter_context(
                    tc.tile_pool(name="av", bufs=2, space="PSUM"))
                ep = s3.enter_context(tc.tile_pool(name="E", bufs=3))

                for h in range(H):
                    t = h // 2
                    r = (h % 2) * 64
                    av = avp.tile([DH + 1, TOK], F32, tag="av", name="avs")
                    prev_e = None
                    for c in range(NC_TOK):
                        S = scp.tile([128, TOK], F32, tag="S", name="Ss")
                        for hf in range(2):
                            nc.tensor.matmul(
                                S[:, hf * 512 : (hf + 1) * 512],
                                k_tiles[t][r : r + 64, c * 128 : (c + 1) * 128],
                                q_tiles[t][r : r + 64, hf * 512 : (hf + 1) * 512],
                                start=True, stop=True)
                        E = ep.tile([128, TOK], BF16, tag="E", name="Es")
                        nc.scalar.activation(
                            out=E, in_=S, func=AF.Exp,
                            bias=am_sb[:, c : c + 1], scale=0.125)
                        if prev_e is not None:
                            pc, pe = prev_e
                            for hf in range(2):
                                nc.tensor.matmul(
                                    av[:, hf * 512 : (hf + 1) * 512],
                                    v_aug[:, pc, h, :],
                                    pe[:, hf * 512 : (hf + 1) * 512],
                                    start=(pc == 0), stop=(pc == NC_TOK - 1))
                        prev_e = (c, E)
                    pc, pe = prev_e
                    for hf in range(2):
                        nc.tensor.matmul(
                            av[:, hf * 512 : (hf + 1) * 512],
                            v_aug[:, pc, h, :],
                            pe[:, hf * 512 : (hf + 1) * 512],
                            start=(pc == 0), stop=(pc == NC_TOK - 1))
                    # evacuate unnormalized output + denominator row together
                    nc.vector.tensor_copy(out=oT[:, h, :], in_=av[:, :])

            if stop_after == "attn":
                dump_out(x2_tiles)
                return nc

            # ======== Phase 3: normalize + output projection ========
            with ExitStack() as s4:
                bcp = s4.enter_context(
                    tc.tile_pool(name="bc", bufs=2, space="PSUM"))
                pop = s4.enter_context(
                    tc.tile_pool(name="po", bufs=4, space="PSUM"))
                rp = s4.enter_context(tc.tile_pool(name="rcp", bufs=1))

                # rcp = exp(-ln(denom)) — same table set as the attention exp
                nc.gpsimd.dma_start(
                    out=den_sb,
                    in_=oT[64:65, :, :].rearrange("p h q -> p (h q)"))
                lnd = rp.tile([8, TOK], F32, tag="lnd", name="lnds")
                nc.scalar.activation(out=lnd, in_=den_sb, func=AF.Ln)
                rcp = rp.tile([8, TOK], BF16, tag="rcp", name="rcps")
                nc.scalar.activation(out=rcp, in_=lnd, func=AF.Exp,
                                     scale=-1.0)
                # matmul operands must sit at base partition {0,32,64}:
                # flatten rcp to one partition row and broadcast from there
                rcp_f = rp.tile([1, H, TOK], BF16, tag="rcpf", name="rcpfs")
                nc.gpsimd.dma_start(out=rcp_f, in_=rcp)

                for h in range(H):
                    bc = bcp.tile([64, TOK], F32, tag="bc", name="bcs")
                    for hf in range(2):
                        nc.tensor.matmul(
                            bc[:, hf * 512 : (hf + 1) * 512],
                            ones8[0:1, :],
                            rcp_f[0:1, h, hf * 512 : (hf + 1) * 512],
                            start=True, stop=True)
                    nc.vector.tensor_mul(
                        out=oTn[:, h, :], in0=oT[0:64, h, :], in1=bc)

                bn2_local = statsp.tile([128, 8], F32, tag="loc_bn2",
                                        name="loc_bn2")
                for o in range(NT):
                    for hf in range(2):
                        p = pop.tile([128, 512], F32, tag="po", name="pos")
                        for h in range(H):
                            nc.tensor.matmul(
                                p,
                                wo_sb[:, h, o * 128 : (o + 1) * 128],
                                oTn[:, h, hf * 512 : (hf + 1) * 512],
                                start=(h == 0), stop=(h == H - 1))
                        # x2 = (proj + bo2) + x
                        nc.vector.scalar_tensor_tensor(
                            out=x2_tiles[o][:, hf * 512 : (hf + 1) * 512],
                            in0=p, scalar=bo2_sb[:, o : o + 1],
                            in1=x_tiles[o][:, hf * 512 : (hf + 1) * 512],
                            op0=ALU.add, op1=ALU.add)
                    _bn_partial(nc, small, "bn2", o, x2_tiles[o], bn2_local)

        if stop_after == "oproj":
            dump_out(x2_tiles)
            return nc

        # ======== Phase 4: BN2 + MLP ========
        with ExitStack() as s5:
            h2p = s5.enter_context(tc.tile_pool(name="h2", bufs=1))
            htp = s5.enter_context(tc.tile_pool(name="ht", bufs=1))
            outp = s5.enter_context(tc.tile_pool(name="outsb", bufs=2))
            pm1 = s5.enter_context(
                tc.tile_pool(name="pm1", bufs=2, space="PSUM"))
            pm2 = s5.enter_context(
                tc.tile_pool(name="pm2", bufs=4, space="PSUM"))

            s2v, t2v = _bn_device(nc, (small, statsp, dram),
                                  g2_sb, be2_sb, "bn2",
                                  collectives=collectives, local=bn2_local)
            warm3 = vec.tile([128, 1], F32, tag="warm3", name="warm3")
            nc.scalar.activation(out=warm3, in_=eps_sb, func=gelu_func)

            h2_tiles = []
            for t in range(NT):
                h2 = h2p.tile([128, TOK], BF16, tag=f"h2_{t}", name=f"h2_{t}")
                nc.vector.tensor_scalar(
                    out=h2, in0=x2_tiles[t],
                    scalar1=s2v[:, t : t + 1], scalar2=t2v[:, t : t + 1],
                    op0=ALU.mult, op1=ALU.add)
                h2_tiles.append(h2)

            ht = htp.tile([128, NH, TOK], BF16, tag="ht", name="hts")
            for o in range(NH):
                p = pm1.tile([128, TOK], F32, tag="pm1", name="pm1s")
                for hf in range(2):
                    for k in range(NT):
                        nc.tensor.matmul(
                            p[:, hf * 512 : (hf + 1) * 512],
                            wm1_sb[:, k, o * 128 : (o + 1) * 128],
                            h2_tiles[k][:, hf * 512 : (hf + 1) * 512],
                            start=(k == 0), stop=(k == NT - 1))
                nc.scalar.activation(
                    out=ht[:, o, :], in_=p, func=gelu_func,
                    bias=b1_sb[:, o : o + 1], scale=1.0)

            for o in range(NT):
                ot = outp.tile([128, TOK], F32, tag="osb", name="osbs")
                for hf in range(2):
                    p = pm2.tile([128, 512], F32, tag="pm2", name="pm2s")
                    for k in range(NH):
                        nc.tensor.matmul(
                            p,
                            wm2_sb[:, k, o * 128 : (o + 1) * 128],
                            ht[:, k, hf * 512 : (hf + 1) * 512],
                            start=(k == 0), stop=(k == NH - 1))
                    nc.vector.scalar_tensor_tensor(
                        out=ot[:, hf * 512 : (hf + 1) * 512],
                        in0=p, scalar=b2_sb[:, o : o + 1],
                        in1=x2_tiles[o][:, hf * 512 : (hf + 1) * 512],
                        op0=ALU.add, op1=ALU.add)
                nc.sync.dma_start(out=OUT[o, :, :], in_=ot)

    return nc


_NC_CACHE = {}


def _get_nc(sim=False):
    if sim not in _NC_CACHE:
        _NC_CACHE[sim] = build(sim=sim)
    return _NC_CACHE[sim]


def make_in_maps(x, mask, Wq, bq, Wk, bk, Wv, bv, Wo, bo, g1, be1, g2, be2,
                 W1, b1m, W2, b2m):
    """Host-side sharding + layout prep. Returns list of per-core input dicts."""
    bf16 = ml_dtypes.bfloat16
    xT = np.ascontiguousarray(np.asarray(x, np.float32).T)      # [512, 8192]
    xTb = np.ascontiguousarray(
        xT.reshape(NT, 128, N_TOTAL)).astype(bf16)

    def wprep(W, nt):
        return np.ascontiguousarray(
            np.asarray(W, np.float32).reshape(nt, 128, -1).transpose(1, 0, 2)
        ).astype(bf16)

    wq = wprep(Wq, NT)
    wk = wprep(Wk, NT)
    wv = wprep(Wv, NT)
    wo = np.ascontiguousarray(
        np.asarray(Wo, np.float32).reshape(H, 64, 512).transpose(1, 0, 2)
    ).astype(bf16)
    wm1 = wprep(W1, NT)
    wm2 = wprep(W2, NH)

    def pp(v, c):
        return np.ascontiguousarray(np.asarray(v, np.float32).reshape(c, 128).T)

    bo2 = np.asarray(bo, np.float32) + (
        np.asarray(bv, np.float32) @ np.asarray(Wo, np.float32))

    shared = {
        "xtb": xTb,
        "wq": wq, "wk": wk, "wv": wv, "wo": wo, "wm1": wm1, "wm2": wm2,
        "bq": pp(bq, 4), "bk": pp(bk, 4), "bo2": pp(bo2, 4),
        "b1m": pp(b1m, 16), "b2m": pp(b2m, 4),
        "g1": pp(g1, 4), "be1": pp(be1, 4), "g2": pp(g2, 4), "be2": pp(be2, 4),
    }
    am_full = np.where(np.asarray(mask, bool), 0.0, -1e9).astype(np.float32)
    in_maps = []
    for core in range(N_CORES):
        sl = xT[:, core * TOK : (core + 1) * TOK]
        m = dict(shared)
        m["xt"] = np.ascontiguousarray(sl.reshape(NT, 128, TOK)).astype(bf16)
        m["am"] = np.ascontiguousarray(am_full[core].reshape(8, 128).T)
        in_maps.append(m)
    return in_maps


_EXEC_CACHE = {}


def _get_executor():
    """Cached PJRT executor for the compiled kernel (same path
    run_bass_kernel_spmd takes under axon, but jitted once and reused)."""
    if "fn" in _EXEC_CACHE:
        return _EXEC_CACHE["fn"]
    import jax
    from jax.sharding import Mesh, PartitionSpec
    from jax.experimental.shard_map import shard_map
    import concourse.bass2jax as b2j

    nc = _get_nc(sim=False)
    b2j.install_neuronx_cc_hook()
    partition_name = (nc.partition_id_tensor.name
                      if nc.partition_id_tensor else None)
    in_names, out_names, out_avals, zero_outs = [], [], [], []
    for alloc in nc.m.functions[0].allocations:
        if not isinstance(alloc, mybir.MemoryLocationSet):
            continue
        name = alloc.memorylocations[0].name
        if alloc.kind == "ExternalInput":
            if name != partition_name:
                in_names.append(name)
        elif alloc.kind == "ExternalOutput":
            out_names.append(name)
            shape = tuple(alloc.tensor_shape)
            dtype = mybir.dt.np(alloc.dtype)
            out_avals.append(jax.core.ShapedArray(shape, dtype))
            zero_outs.append(np.zeros(shape, dtype))
    n_params = len(in_names)
    all_names = in_names + out_names
    if partition_name is not None:
        all_names = all_names + [partition_name]

    def _body(*args):
        operands = list(args)
        if partition_name is not None:
            operands.append(b2j.partition_id_tensor())
        return tuple(b2j._bass_exec_p.bind(
            *operands,
            out_avals=tuple(out_avals),
            in_names=tuple(all_names),
            out_names=tuple(out_names),
            lowering_input_output_aliases=(),
            sim_require_finite=True,
            sim_require_nnan=True,
            nc=nc,
        ))

    devices = jax.devices()[:N_CORES]
    mesh = Mesh(np.asarray(devices), ("core",))
    n_out = len(out_names)
    sharded = jax.jit(
        shard_map(_body, mesh=mesh,
                  in_specs=(PartitionSpec("core"),) * (n_params + n_out),
                  out_specs=(PartitionSpec("core"),) * n_out,
                  check_rep=False),
        keep_unused=True)

    def run(in_maps):
        per_core = [[np.asarray(m[nm]) for nm in in_names] for m in in_maps]
        concat_in = [
            np.concatenate([per_core[c][i] for c in range(N_CORES)], axis=0)
            for i in range(n_params)]
        concat_zeros = [
            np.zeros((N_CORES * z.shape[0], *z.shape[1:]), z.dtype)
            for z in zero_outs]
        out_arrs = sharded(*concat_in, *concat_zeros)
        return [
            {name: np.asarray(out_arrs[i]).reshape(
                N_CORES, *out_avals[i].shape)[c]
             for i, name in enumerate(out_names)}
            for c in range(N_CORES)]

    _EXEC_CACHE["fn"] = run
    return run


def gather_out(results):
    """results: list of per-core dicts with 'outt' [4, 128, 1024] -> [8192, 512]."""
    outs = []
    for core in range(N_CORES):
        oT = results[core]["outt"].reshape(ED, TOK)   # [512, 1024]
        outs.append(oT.T)                             # [1024, 512]
    return np.concatenate(outs, axis=0).astype(np.float32)


def kernel(**inputs) -> np.ndarray:
    inputs = dict(inputs)
    inputs.pop("b", None)
    inputs.pop("gs", None)
    in_maps = make_in_maps(**inputs)
    run = _get_executor()
    return gather_out(run(in_maps))
